# revision 4
# baseline (speedup 1.0000x reference)
"""Sparse multi-head attention (B=4, S=2048, F=512, H=8, D=64) on 8 trn2 cores.

Sharding: core c handles batch b = c % 4 and heads [hg*4, hg*4+4) with
hg = c // 4.  Per-core engine-balanced design:

PE (all matmuls, "scores transposed" layout, head PAIRS):
  - QK projection: W^T-stationary, M=128 (head j0 in partitions 0:64,
    j1 in 64:128), fp16.  K-bias dropped (softmax-invariant); Q-bias
    carries the pair scale.
  - V produced directly in [t, d] layout: X^T-block stationary
    [128f, 128t], W_v moving [128f, 4*64] -> psum [t, 256], so no PE
    transposes and no per-block DVE copies.
  - scores: 2 heads run CONCURRENTLY as k=64 row-tiles (tile_position
    (0,0) / (64,0) auto-derived from base partitions).
  - attn*V: lhsT = V[t,64]; the softmax denominator rides col-group 2
    as a concurrent m=1 ones-matmul (tile_position (0,64)) on the same
    rhs stream, so it costs no extra PE time.
  - V-bias dropped: softmax weights sum to 1 => host adds b_v to Y.

exp (the elementwise whale, 16.8M elems/core) is SPLIT:
  - ACT tbs: ACT exp (scale=1/A) from psum -> bf16, then mask-mult
    (bf16 x {0,1}) on DVE or GPSIMD.
  - SCHR tbs: single DVE op: i16 = convert(psum + mask_i16[t,s]) where
    psum = A*score, A = 128*log2(e) (folded into Wq/Wk on host).  The
    i16 IS the bf16 bit pattern of exp(score) (Schraudolph); masked
    entries get -32768 -> convert saturates/lands in tiny-negative
    bf16 ~ -0.0.  One op does exp+mask.  ~0.8% extra rel err.

Host divides by the denominator row and interleaves heads.
"""

import sys

for _p in ("/opt/trn_rl_repo", "/root/.axon_site/_ro/trn_rl_repo"):
    if _p not in sys.path:
        sys.path.insert(0, _p)

from contextlib import ExitStack

import ml_dtypes
import numpy as np

import concourse.bacc as bacc
import concourse.tile as tile
from concourse import bass_utils, mybir

B, S, F, H, D = 4, 2048, 512, 8, 64
HPC = H // 2  # heads per core (4): 2 head-groups x 4 batches = 8 cores
NPAIR = HPC // 2  # head pairs per core (2)
N_CORES = 8
NF = F // 128  # 4 f-chunks of 128
NT = S // 128  # 16 t-blocks

# exp path split + Schraudolph constants
A_SCALE = 128.0 * np.log2(np.e)  # psum = A_SCALE * score
SCHR_TBS = frozenset((1, 3, 5, 8, 10, 12, 14))  # 7/16 t-blocks on DVE path
C_UNMASK = 16249  # 16256 + delta, delta = -7 (var-min ~ -7.3)
C_MASK = -32768
# of the ACT-path (tb, head) mask-mults, which go to gpsimd
GPS_OPC = frozenset((0, 4, 7, 11, 15))  # tb values; gpsimd takes head 1

F32 = mybir.dt.float32
BF16 = mybir.dt.bfloat16
FP16 = mybir.dt.float16
I16 = mybir.dt.int16
AF = mybir.ActivationFunctionType
ALU = mybir.AluOpType


def build_nc():
    nc = bacc.Bacc(
        "TRN2", target_bir_lowering=False, debug=False, num_devices=N_CORES
    )
    xt_d = nc.dram_tensor("xt", [F, S], FP16, kind="ExternalInput").ap()
    # mask, 2 bytes/elem: bf16 {1,0} bits for ACT tbs, {C_UNMASK, C_MASK}
    # i16 for SCHR tbs
    mk_d = nc.dram_tensor("msk", [S, S], I16, kind="ExternalInput").ap()
    # per pair m: [q_j0|q_j1|k_j0|k_j1] (256 cols); q cols pre-scaled
    wqk_d = nc.dram_tensor("wqk", [F, NPAIR * 256], FP16, kind="ExternalInput").ap()
    wv_d = nc.dram_tensor("wv", [F, HPC * 64], FP16, kind="ExternalInput").ap()
    bq_d = nc.dram_tensor("bq", [128, NPAIR], F32, kind="ExternalInput").ap()
    yt_d = nc.dram_tensor("yt", [HPC, 65, S], F32, kind="ExternalOutput").ap()
    junk_d = nc.dram_tensor("junk", [64, 512], F32)  # warmup sink (Internal)

    with ExitStack() as ctx:
        tc = ctx.enter_context(tile.TileContext(nc))
        const = ctx.enter_context(tc.tile_pool(name="const", bufs=1))

        wqk_sb = const.tile([128, NF, NPAIR * 256], FP16)
        nc.sync.dma_start(wqk_sb[:], wqk_d.rearrange("(c p) n -> p c n", p=128))
        bq_sb = const.tile([128, NPAIR], F32)
        nc.sync.dma_start(bq_sb[:], bq_d)

        xt_sb = const.tile([128, NF, S], FP16)
        xt_r = xt_d.rearrange("(c p) s -> p c s", p=128)
        wv_sb = const.tile([128, NF, HPC * 64], FP16)
        # s-half-sliced so the first projection blocks can start after ~1MB
        for sh in range(2):
            hsl = slice(sh * (S // 2), (sh + 1) * (S // 2))
            for c in range(NF):
                nc.sync.dma_start(xt_sb[:, c, hsl], xt_r[:, c, hsl])
            if sh == 0:
                nc.sync.dma_start(
                    wv_sb[:], wv_d.rearrange("(c p) n -> p c n", p=128)
                )
        mk_sb = const.tile([128, NT, S], I16)
        mk_r = mk_d.rearrange("(t p) s -> p t s", p=128)
        for i in range(8):
            g = NT // 8
            nc.sync.dma_start(
                mk_sb[:, i * g : (i + 1) * g, :], mk_r[:, i * g : (i + 1) * g, :]
            )

        ones_sb = const.tile([128, 1], BF16)
        nc.vector.memset(ones_sb[:], 1.0)
        # V in [t, d] layout: per t-block, 4 heads side by side
        v_sb = const.tile([128, NT, HPC * 64], BF16)
        # q/k per pair: [128, S] fp16, head j0 rows 0:64, j1 rows 64:128
        qt_sb = [const.tile([128, S], FP16, name=f"qt{m}") for m in range(NPAIR)]
        kt_sb = [const.tile([128, S], FP16, name=f"kt{m}") for m in range(NPAIR)]

        e_pool = ctx.enter_context(tc.tile_pool(name="e", bufs=4))
        y_pool = ctx.enter_context(tc.tile_pool(name="y", bufs=2))
        ps = ctx.enter_context(tc.tile_pool(name="ps", bufs=1, space="PSUM"))

        def ps_tile(tag):
            return ps.tile([128, 1024], F32, tag=tag, name=f"ps_{tag}")

        # --- PE warmup: junk matmuls with a full 128x128 stationary and no
        # DMA dependency, so the HAM clock-gate opens to K=8/8 before the
        # real work arrives.
        NWU = 28
        wu = const.tile([128, 512], BF16)
        nc.vector.memset(wu[:], 0.0)
        pw = ps_tile("a")
        for i in range(NWU):
            nc.tensor.matmul(
                pw[:, 0:512],
                wu[:, 0:128],
                wu[:],
                start=(i == 0),
                stop=(i == NWU - 1),
            )
        wu_out = const.tile([64, 512], F32)
        nc.vector.tensor_copy(wu_out[:], pw[0:64, 0:512])
        nc.sync.dma_start(junk_d.ap(), wu_out[:])

        # --- Phase 0a: QK projection per pair (4-deep psum rotation).
        tags = ("a", "b", "ya", "yb")
        ti = 0
        for m in range(NPAIR):
            for sq in range(S // 512):
                ssl = slice(sq * 512, (sq + 1) * 512)
                for kind in range(2):  # 0 = q, 1 = k
                    wsl = slice(m * 256 + kind * 128, m * 256 + (kind + 1) * 128)
                    pp = ps_tile(tags[ti % 4])
                    ti += 1
                    for c in range(NF):
                        nc.tensor.matmul(
                            pp[:, 0:512],
                            wqk_sb[:, c, wsl],
                            xt_sb[:, c, ssl],
                            start=(c == 0),
                            stop=(c == NF - 1),
                        )
                    if kind == 0:
                        # psum + per-partition q bias, on DVE
                        nc.vector.tensor_scalar(
                            qt_sb[m][:, ssl],
                            pp[:, 0:512],
                            bq_sb[:, m : m + 1],
                            None,
                            op0=ALU.add,
                        )
                    else:
                        # plain copy on ACT (bias-free K)
                        nc.scalar.activation(
                            kt_sb[m][:, ssl], pp[:, 0:512], AF.Copy
                        )

        # --- Phase 0b: V via X^T-stationary (V lands directly as [t, d]).
        for tb in range(NT):
            tsl = slice(tb * 128, (tb + 1) * 128)
            vp = ps_tile(tags[ti % 4])
            ti += 1
            for c in range(NF):
                nc.tensor.matmul(
                    vp[:, 0 : HPC * 64],
                    xt_sb[:, c, tsl],
                    wv_sb[:, c, :],
                    start=(c == 0),
                    stop=(c == NF - 1),
                )
            nc.scalar.activation(v_sb[:, tb, :], vp[:, 0 : HPC * 64], AF.Copy)

        # --- Attention: per (pair, query-half).  Software-pipelined so the
        # PE FIFO holds scores(tb+1) ahead of attn*V(tb).
        for m in range(NPAIR):
            qt, kt = qt_sb[m], kt_sb[m]
            for qh in range(2):
                py = [
                    ps.tile([65, 1024], F32, tag=t, name=f"py_{t}")
                    for t in ("ya", "yb")
                ]
                sc = {}

                def emit_scores(tb):
                    tsl = slice(tb * 128, (tb + 1) * 128)
                    p0 = ps_tile("a")
                    p1 = ps_tile("b")
                    for qb in range(2):
                        qsl = slice(qh * 1024 + qb * 512, qh * 1024 + (qb + 1) * 512)
                        osl = slice(qb * 512, (qb + 1) * 512)
                        nc.tensor.matmul(
                            p0[:, osl], kt[0:64, tsl], qt[0:64, qsl],
                            start=True, stop=True,
                        )
                        nc.tensor.matmul(
                            p1[:, osl], kt[64:128, tsl], qt[64:128, qsl],
                            start=True, stop=True,
                        )
                    sc[tb] = (p0, p1)

                emit_scores(0)
                for tb in range(NT):
                    if tb + 1 < NT:
                        emit_scores(tb + 1)
                    p2 = sc.pop(tb)
                    msl = slice(qh * 1024, (qh + 1) * 1024)
                    es = []
                    for j in range(2):
                        e = e_pool.tile([128, 1024], BF16, tag=f"e{j}", name="e")
                        if tb in SCHR_TBS:
                            # exp+mask fused: i16 out IS the bf16 pattern
                            nc.vector.tensor_tensor(
                                e[:].bitcast(I16),
                                p2[j][:],
                                mk_sb[:, tb, msl],
                                op=ALU.add,
                            )
                        else:
                            nc.scalar.activation(
                                e[:], p2[j][:], AF.Exp, scale=1.0 / A_SCALE
                            )
                            eng = (
                                nc.gpsimd
                                if (tb in GPS_OPC and j == 1)
                                else nc.vector
                            )
                            eng.tensor_tensor(
                                e[:],
                                e[:],
                                mk_sb[:, tb, msl].bitcast(BF16),
                                op=ALU.mult,
                            )
                        es.append(e)
                    for j in range(2):
                        vsl = slice((2 * m + j) * 64, (2 * m + j + 1) * 64)
                        for qb in range(2):
                            osl = slice(qb * 512, (qb + 1) * 512)
                            nc.tensor.matmul(
                                py[j][0:64, osl],
                                v_sb[:, tb, vsl],
                                es[j][:, osl],
                                start=(tb == 0),
                                stop=(tb == NT - 1),
                            )
                            nc.tensor.matmul(
                                py[j][64:65, osl],
                                ones_sb[:],
                                es[j][:, osl],
                                start=(tb == 0),
                                stop=(tb == NT - 1),
                            )
                for j in range(2):
                    y_sb = y_pool.tile([65, 1024], F32, tag=f"y{j}", name="y_sb")
                    nc.vector.tensor_copy(y_sb[:], py[j][:])
                    nc.sync.dma_start(
                        yt_d[2 * m + j, :, qh * 1024 : (qh + 1) * 1024], y_sb[:]
                    )

    nc.compile()
    return nc


_NC_CACHE = {}


def _get_nc():
    if "nc" not in _NC_CACHE:
        _NC_CACHE["nc"] = build_nc()
    return _NC_CACHE["nc"]


def make_in_maps(X, A, W, b):
    X = np.ascontiguousarray(np.asarray(X), dtype=np.float32)
    A = np.asarray(A)
    W = np.ascontiguousarray(np.asarray(W), dtype=np.float32)
    b = np.ascontiguousarray(np.asarray(b), dtype=np.float32)
    _NC_CACHE["b"] = b
    alpha = np.float32(np.sqrt(A_SCALE / np.sqrt(np.float32(H))))
    d = np.arange(D)

    xts = [np.ascontiguousarray(X[bb].T).astype(np.float16) for bb in range(B)]
    # mask [t, s]; bf16 {1, 0} bit patterns for ACT tbs, schraudolph
    # additive constants for SCHR tbs -- both as raw int16
    one_bits = np.float32(1.0).astype(ml_dtypes.bfloat16).view(np.int16)
    msks = []
    for bb in range(B):
        mT = np.ascontiguousarray(A[bb].T)
        mk = np.empty((S, S), np.int16)
        for tb in range(NT):
            sl = slice(tb * 128, (tb + 1) * 128)
            if tb in SCHR_TBS:
                mk[sl] = np.where(mT[sl], C_UNMASK, C_MASK).astype(np.int16)
            else:
                mk[sl] = np.where(mT[sl], one_bits, 0).astype(np.int16)
        msks.append(mk)

    # per head-group weight/bias packs (head-pair layout)
    packs = []
    for hg in range(2):
        wqk = np.empty((F, NPAIR * 256), np.float32)
        wv = np.empty((F, HPC * 64), np.float32)
        bq = np.empty((128, NPAIR), np.float32)
        for m in range(NPAIR):
            for half in range(2):
                h = hg * HPC + 2 * m + half
                qc = d * 24 + h
                kc = d * 24 + 8 + h
                vc = d * 24 + 16 + h
                c0 = m * 256 + half * 64
                wqk[:, c0 : c0 + 64] = W[:, qc] * alpha
                wqk[:, c0 + 128 : c0 + 192] = W[:, kc] * alpha
                wv[:, (2 * m + half) * 64 : (2 * m + half + 1) * 64] = W[:, vc]
                rsl = slice(64 * half, 64 * (half + 1))
                bq[rsl, m] = b[qc] * alpha
        packs.append((wqk.astype(np.float16), wv.astype(np.float16), bq))

    in_maps = []
    for c in range(N_CORES):
        bb = c % B
        hg = c // B
        wqk, wv, bq = packs[hg]
        in_maps.append(
            {
                "xt": xts[bb],
                "msk": msks[bb],
                "wqk": wqk,
                "wv": wv,
                "bq": bq,
            }
        )
    return in_maps


def assemble_output(results):
    b_full = _NC_CACHE["b"]
    Y = np.empty((B, S, D * H), np.float32)
    Yv = Y.reshape(B, S, D, H)
    d = np.arange(D)
    for c in range(N_CORES):
        bb = c % B
        hg = c // B
        yt = results[c]["yt"]  # [HPC, 65, S]
        for j in range(HPC):
            h = hg * HPC + j
            bv = b_full[d * 24 + 16 + h]
            Yv[bb, :, :, h] = (yt[j, 0:64, :] / yt[j, 64:65, :]).T + bv[None, :]
    return Y


def kernel(X, A, W, b):
    nc = _get_nc()
    in_maps = make_in_maps(X, A, W, b)
    res = bass_utils.run_bass_kernel_spmd(
        nc, in_maps, core_ids=list(range(N_CORES))
    ).results
    return assemble_output(res)
